# revision 1
# baseline (speedup 1.0000x reference)
"""ComplEx scoring kernel for 8 Trainium2 NeuronCores.

Math: score[b, e] = Re(<h_b * r_b, conj(ent_e)>) with h = ent_emb[triples[:,0]],
r = rel_emb[triples[:,1]].  Writing ans_b = concat(re_h*re_r - im_h*im_r,
re_h*im_r + im_h*re_r) (shape [B, 512]), the score is exactly
score = ans @ ent_emb.T  — one [1024, 512] x [512, 200000] GEMM.

Strategy (vocab/tensor parallel, per the entity axis):
  - host: tiny gather + complex multiply -> ans  (microseconds)
  - shard ent_emb rows 8 ways (25000/core, zero-padded to 25088 = 49*512),
    pre-transposed + bf16-cast on host so the device streams contiguous
    [K=512, E] tiles
  - each core: score_shard[1024, 25088] f32 = ansT.T @ entT via PE-array
    matmuls (bf16 in, fp32 PSUM accumulate), memory-bound streaming
  - host: concatenate the 8 column slabs, drop padding
"""

import numpy as np
import ml_dtypes

NCORES = 8
NUM_ENT = 200000
EMB = 512
B = 1024
SHARD = NUM_ENT // NCORES      # 25000 entities per core
NTILE = 512                    # matmul moving free dim == one PSUM bank
TPG = 7                        # 512-tiles per DMA group
GN = NTILE * TPG               # 3584 entities per group
NGROUPS = 7
SHARD_PAD = GN * NGROUPS       # 25088
KCH = EMB // 128               # 4 contraction chunks
MCH = B // 128                 # 8 batch chunks

_NC = None

# score values are ~1e-5 — subnormal in fp16.  Pre-scaling ans by 2**16 on
# the host puts the device-side scores in fp16's normal range, so the output
# can be stored/DMA'd as fp16 (half the write traffic); the host unscales.
OUT_SCALE = 2.0 ** 16


def _build_nc():
    import concourse.bacc as bacc
    import concourse.bass as bass
    import concourse.tile as tile
    from concourse import mybir

    ts, ds = bass.ts, bass.ds
    bf16 = mybir.dt.bfloat16
    f16 = mybir.dt.float16
    f32 = mybir.dt.float32

    nc = bacc.Bacc("TRN2", target_bir_lowering=False, debug=False)
    ansT = nc.dram_tensor("ansT", [EMB, B], bf16, kind="ExternalInput")
    entT = nc.dram_tensor("entT", [EMB, SHARD_PAD], bf16, kind="ExternalInput")
    score = nc.dram_tensor("score", [B, SHARD_PAD], f16, kind="ExternalOutput")

    with tile.TileContext(nc) as tc:
        with tc.tile_pool(name="const", bufs=1) as const_pool, \
             tc.tile_pool(name="entp", bufs=3 * KCH) as ent_pool, \
             tc.tile_pool(name="outp", bufs=3) as out_pool, \
             tc.tile_pool(name="ps", bufs=8, space="PSUM") as psum_pool:

            def load_group(g):
                # one tile per k-chunk so a matmul only waits for its own DMA
                tiles = []
                for k in range(KCH):
                    t = ent_pool.tile([128, GN], bf16, name="ent_sb")
                    nc.sync.dma_start(t[:], entT[ts(k, 128), ds(g * GN, GN)])
                    tiles.append(t)
                return tiles

            # startup: ansT first (small), then group 0 split into per-tile
            # DMAs in PE consume order so early matmuls start as soon as
            # their slice lands (all on the SP queue, in order)
            ansT_sb = const_pool.tile([128, KCH, B], bf16, name="ansT_sb")
            for k in range(KCH):
                nc.sync.dma_start(ansT_sb[:, k], ansT[ts(k, 128), :])
            # t-major issue order matches the first block's t-outer consume
            # order, so the first psum tile only waits for ~4 small DMAs
            ent_sb0 = [ent_pool.tile([128, GN], bf16, name="ent_sb")
                       for _ in range(KCH)]
            for tt in range(TPG):
                for k in range(KCH):
                    nc.sync.dma_start(ent_sb0[k][:, ts(tt, NTILE)],
                                      entT[ts(k, 128), ds(tt * NTILE, NTILE)])

            # gpsimd (Pool) cannot read PSUM on TRN2 — copyback on DVE + Act
            copy_engines = [nc.vector, nc.scalar]
            ci = 0
            ent_tiles = {0: ent_sb0}
            for g in range(NGROUPS):
                # prefetch next group's tiles ahead of this group's output DMAs
                # so the SP engine's in-order stream doesn't serialize them
                if g + 1 < NGROUPS:
                    ent_tiles[g + 1] = load_group(g + 1)
                ent_sb = ent_tiles.pop(g)
                for m in range(MCH):
                    pss = [psum_pool.tile([128, NTILE], f32, name="pst")
                           for _ in range(TPG)]
                    out_sb = out_pool.tile([128, GN], f16, name="out_sb")
                    # first block: t-outer so the PE starts on partial data;
                    # last block: t-outer so the drain overlaps the matmuls
                    t_outer = ((g == NGROUPS - 1) and (m == MCH - 1)) or \
                              (g == 0 and m == 0)

                    def copyback(t):
                        nonlocal ci
                        eng = copy_engines[ci % len(copy_engines)]
                        ci += 1
                        if eng is nc.scalar:
                            eng.copy(out_sb[:, ts(t, NTILE)], pss[t][:])
                        else:
                            eng.tensor_copy(out=out_sb[:, ts(t, NTILE)],
                                            in_=pss[t][:])

                    if not t_outer:
                        # k outer / tile inner: stationary weights switch once
                        # per TPG matmuls instead of every matmul
                        for k in range(KCH):
                            lhsT = ansT_sb[:, k, ts(m, 128)]
                            for t in range(TPG):
                                nc.tensor.matmul(
                                    pss[t][:],
                                    lhsT,
                                    ent_sb[k][:, ts(t, NTILE)],
                                    start=(k == 0),
                                    stop=(k == KCH - 1),
                                )
                        for t in range(TPG):
                            copyback(t)
                        # two half-width output DMAs so the drain starts as
                        # soon as the first copies land
                        h0 = 4 * NTILE
                        nc.sync.dma_start(score[ts(m, 128), ds(g * GN, h0)],
                                          out_sb[:, :h0])
                        nc.sync.dma_start(
                            score[ts(m, 128), ds(g * GN + h0, GN - h0)],
                            out_sb[:, h0:])
                    else:
                        # t-outer: each psum tile finishes its accumulation
                        # early; interleave copies + small DMAs so the drain
                        # (or warm-up) overlaps the remaining matmuls
                        for t in range(TPG):
                            for k in range(KCH):
                                nc.tensor.matmul(
                                    pss[t][:],
                                    ansT_sb[:, k, ts(m, 128)],
                                    ent_sb[k][:, ts(t, NTILE)],
                                    start=(k == 0),
                                    stop=(k == KCH - 1),
                                )
                            copyback(t)
                            if t % 2 == 1:
                                nc.sync.dma_start(
                                    score[ts(m, 128),
                                          ds(g * GN + (t - 1) * NTILE, 2 * NTILE)],
                                    out_sb[:, ds((t - 1) * NTILE, 2 * NTILE)])
                        nc.sync.dma_start(
                            score[ts(m, 128), ds(g * GN + (TPG - 1) * NTILE, NTILE)],
                            out_sb[:, ds((TPG - 1) * NTILE, NTILE)])
    nc.compile()
    return nc


def _get_nc():
    global _NC
    if _NC is None:
        _NC = _build_nc()
    return _NC


def _pmap(fn, n):
    from concurrent.futures import ThreadPoolExecutor
    with ThreadPoolExecutor(max_workers=n) as ex:
        list(ex.map(fn, range(n)))


def prepare_in_maps(triples, ent_emb, rel_emb):
    triples = np.asarray(triples)
    ent_emb = np.asarray(ent_emb, dtype=np.float32)
    rel_emb = np.asarray(rel_emb, dtype=np.float32)

    d = EMB // 2
    h = ent_emb[triples[:, 0].astype(np.int64)]
    r = rel_emb[triples[:, 1].astype(np.int64)]
    re_h, im_h = h[:, :d], h[:, d:]
    re_r, im_r = r[:, :d], r[:, d:]
    ans = np.empty((B, EMB), np.float32)
    ans[:, :d] = re_h * re_r - im_h * im_r
    ans[:, d:] = re_h * im_r + im_h * re_r
    ans *= np.float32(OUT_SCALE)
    ansT_bf = np.ascontiguousarray(ans.T).astype(ml_dtypes.bfloat16)

    ent_bf = np.empty(ent_emb.shape, dtype=ml_dtypes.bfloat16)
    shards = np.empty((NCORES, EMB, SHARD_PAD), dtype=ml_dtypes.bfloat16)

    def _cast(c):
        s = slice(c * SHARD, (c + 1) * SHARD)
        ent_bf[s] = ent_emb[s]

    def _shard(c):
        shards[c, :, :SHARD] = ent_bf[c * SHARD:(c + 1) * SHARD].T
        shards[c, :, SHARD:] = 0

    _pmap(_cast, NCORES)
    _pmap(_shard, NCORES)
    return [{"ansT": ansT_bf, "entT": shards[c]} for c in range(NCORES)]


def run_raw(in_maps, trace=False):
    from concourse import bass_utils
    return bass_utils.run_bass_kernel_spmd(
        _get_nc(), in_maps, core_ids=list(range(NCORES)), trace=trace
    )


def assemble(results):
    out = np.empty((B, NUM_ENT), np.float32)
    inv = np.float32(1.0 / OUT_SCALE)

    def _one(c):
        sh = results[c]["score"][:, :SHARD].astype(np.float32)
        sh *= inv
        out[:, c * SHARD:(c + 1) * SHARD] = sh

    _pmap(_one, NCORES)
    return out


def kernel(triples, ent_emb, rel_emb):
    in_maps = prepare_in_maps(triples, ent_emb, rel_emb)
    res = run_raw(in_maps)
    return assemble(res.results)



# revision 5
# speedup vs baseline: 1.1297x; 1.1297x over previous
"""ComplEx scoring kernel for 8 Trainium2 NeuronCores.

Math: score[b, e] = Re(<h_b * r_b, conj(ent_e)>) with h = ent_emb[triples[:,0]],
r = rel_emb[triples[:,1]].  Writing ans_b = concat(re_h*re_r - im_h*im_r,
re_h*im_r + im_h*re_r) (shape [B, 512]), the score is exactly
score = ans @ ent_emb.T  — one [1024, 512] x [512, 200000] GEMM.

Strategy (vocab/tensor parallel on the entity axis, 25000 entities/core,
padded to 25088 = 49x512 columns):
  - host: tiny gather + complex multiply -> ans  (microseconds)
  - the GEMM is TensorE-bound (26.3 GFLOP/core vs 78.6 TF/s bf16), so the
    entity axis is split into a bf16 part (38 tiles of 512) and an fp8-e4m3
    DoubleRow part (11 tiles) that runs the PE at 2 MACs/cell/cycle.  The
    fp8 fraction (22% of entities) is sized so the fp8 quantization noise
    (3.75e-2 on those columns) keeps the global rel err ~1.8e-2 (<2e-2).
  - per core: score_bf16[1024, 19456] + score_fp8[1024, 5632], both f16,
    PE pre-warmed with dummy matmuls so the HAM clock gate opens during the
    DMA preamble instead of 3.4us into real work.
  - host: concatenate the 8 column slabs, per-region unscale, drop padding
"""

import numpy as np
import ml_dtypes

NCORES = 8
NUM_ENT = 200000
EMB = 512
B = 1024
SHARD = NUM_ENT // NCORES      # 25000 entities per core
NTILE = 512                    # matmul moving free dim == one PSUM bank
NB_TILES = 38                  # bf16 512-tiles per core
NF_TILES = 11                  # fp8 512-tiles per core
NB = NB_TILES * NTILE          # 19456 bf16 columns
NF = NF_TILES * NTILE          # 5632 fp8 columns (5544 real + 88 pad)
SHARD_PAD = NB + NF            # 25088
NF_REAL = SHARD - NB           # 5544 real entities in the fp8 region
GROUPS = [7, 7, 7, 7, 7, 3]    # bf16 tile groups (DMA/reuse granularity)
GN_FULL = 7 * NTILE            # 3584
KCH = EMB // 128               # 4 contraction chunks
MCH = B // 128                 # 8 batch chunks

_NC = None
_SCALES = {}

# bf16-path score values are ~1e-5 — subnormal in fp16.  Pre-scaling ans by
# 2**16 on the host puts the device-side scores in fp16's normal range; the
# host unscales.  The fp8 path has its own scales (s_a, s_e) chosen at prep
# time so quantized inputs sit in e4m3's range and scores fit fp16.
OUT_SCALE = 2.0 ** 16


def _build_nc():
    import concourse.bacc as bacc
    import concourse.bass as bass
    import concourse.tile as tile
    from concourse import mybir

    ts, ds = bass.ts, bass.ds
    bf16 = mybir.dt.bfloat16
    f16 = mybir.dt.float16
    f8 = mybir.dt.float8e4
    f32 = mybir.dt.float32
    DR = mybir.MatmulPerfMode.DoubleRow

    nc = bacc.Bacc("TRN2", target_bir_lowering=False, debug=False)
    ansT = nc.dram_tensor("ansT", [EMB, B], bf16, kind="ExternalInput")
    ans8 = nc.dram_tensor("ans8", [128, KCH, B], f8, kind="ExternalInput")
    entT = nc.dram_tensor("entT", [EMB, NB], bf16, kind="ExternalInput")
    ent8 = nc.dram_tensor("ent8", [128, KCH, NF], f8, kind="ExternalInput")
    score = nc.dram_tensor("score", [B, SHARD_PAD], f16, kind="ExternalOutput")

    with tile.TileContext(nc) as tc:
        with tc.tile_pool(name="entp", bufs=3 * KCH) as ent_pool, \
             tc.tile_pool(name="outp", bufs=4) as out_pool, \
             tc.tile_pool(name="out8p", bufs=2) as out8_pool, \
             tc.tile_pool(name="ps", bufs=8, space="PSUM") as psum_pool:

            _frees = []
            ansT_sb, _f = tc.tile([128, KCH, B], bf16, name="ansT_sb")
            _frees.append(_f)
            ans8_sb, _f = tc.tile([128, KCH, B], f8, name="ans8_sb")
            _frees.append(_f)
            ent8_sb, _f = tc.tile([128, KCH, NF], f8, name="ent8_sb")
            _frees.append(_f)
            wup, _f = tc.tile([128, 640], bf16, name="wup")
            _frees.append(_f)

            # PE pre-warm: ~4us of zero matmuls so the HAM clock gate opens
            # during the preamble/DMA wait; real matmuls then run at 2.4 GHz
            # from the first instruction.
            nc.vector.memset(wup[:], 0)
            wps = psum_pool.tile([128, NTILE], f32, name="pst")
            for i in range(10):
                nc.tensor.matmul(wps[:], wup[:, ds(0, 128)],
                                 wup[:, ds(128, 512)],
                                 start=(i == 0), stop=(i == 9))

            def load_group(g, gcol):
                # one tile per k-chunk so a matmul only waits for its own DMA
                gn = GROUPS[g] * NTILE
                tiles = []
                for k in range(KCH):
                    t = ent_pool.tile([128, GN_FULL], bf16, name="ent_sb")
                    nc.sync.dma_start(t[:, ds(0, gn)],
                                      entT[ts(k, 128), ds(gcol, gn)])
                    tiles.append(t)
                return tiles

            # startup: interleave ansT chunks with group-0 tile-0 slices so
            # the first matmul only waits for ~0.4 MB, then t-major order
            # matches the first block's consume order
            ent_sb0 = [ent_pool.tile([128, GN_FULL], bf16, name="ent_sb")
                       for _ in range(KCH)]
            nc.sync.dma_start(ansT_sb[:, 0], ansT[ts(0, 128), :])
            for k in range(KCH):
                nc.sync.dma_start(ent_sb0[k][:, ts(0, NTILE)],
                                  entT[ts(k, 128), ds(0, NTILE)])
            for k in range(1, KCH):
                nc.sync.dma_start(ansT_sb[:, k], ansT[ts(k, 128), :])
            for tt in range(1, GROUPS[0]):
                for k in range(KCH):
                    nc.sync.dma_start(ent_sb0[k][:, ts(tt, NTILE)],
                                      entT[ts(k, 128), ds(tt * NTILE, NTILE)])

            # gpsimd (Pool) cannot read PSUM on TRN2 — copyback on DVE + Act
            copy_engines = [nc.vector, nc.scalar]
            ci = 0

            def copyback(dst, ps):
                nonlocal ci
                eng = copy_engines[ci % len(copy_engines)]
                ci += 1
                if eng is nc.scalar:
                    eng.copy(dst, ps)
                else:
                    eng.tensor_copy(out=dst, in_=ps)

            ent_tiles = {0: ent_sb0}
            gcols = np.cumsum([0] + [gs * NTILE for gs in GROUPS]).tolist()
            for g, gsz in enumerate(GROUPS):
                gn = gsz * NTILE
                col = gcols[g]
                # prefetch next group's tiles ahead of this group's output
                # DMAs so the SP engine's in-order stream doesn't serialize
                if g + 1 < len(GROUPS):
                    ent_tiles[g + 1] = load_group(g + 1, gcols[g + 1])
                if g == 1:
                    # fp8 operands, needed only at the very end
                    nc.sync.dma_start(ans8_sb[:], ans8[:, :, :])
                    nc.sync.dma_start(ent8_sb[:, ds(0, 2)], ent8[:, ds(0, 2), :])
                    nc.sync.dma_start(ent8_sb[:, ds(2, 2)], ent8[:, ds(2, 2), :])
                ent_sb = ent_tiles.pop(g)

                if g == 0:
                    # warm-up: interleave m0+m1 t-outer so PE consumption
                    # (2x4 matmuls per tile) outpaces the tile DMA stream and
                    # never starves while group 0 lands
                    outs = [out_pool.tile([128, GN_FULL], f16, name="out_sb")
                            for _ in range(2)]
                    for t in range(gsz):
                        for m in range(2):
                            ps = psum_pool.tile([128, NTILE], f32, name="pst")
                            for k in range(KCH):
                                nc.tensor.matmul(
                                    ps[:], ansT_sb[:, k, ts(m, 128)],
                                    ent_sb[k][:, ts(t, NTILE)],
                                    start=(k == 0), stop=(k == KCH - 1))
                            copyback(outs[m][:, ts(t, NTILE)], ps[:])
                        if t % 2 == 1:
                            for m in range(2):
                                nc.sync.dma_start(
                                    score[ts(m, 128), ds(col + (t - 1) * NTILE, 2 * NTILE)],
                                    outs[m][:, ds((t - 1) * NTILE, 2 * NTILE)])
                    for m in range(2):
                        nc.sync.dma_start(
                            score[ts(m, 128), ds(col + (gsz - 1) * NTILE, NTILE)],
                            outs[m][:, ds((gsz - 1) * NTILE, NTILE)])
                    ms = range(2, MCH)
                else:
                    ms = range(MCH)

                for m in ms:
                    pss = [psum_pool.tile([128, NTILE], f32, name="pst")
                           for _ in range(gsz)]
                    out_sb = out_pool.tile([128, GN_FULL], f16, name="out_sb")
                    # k outer: keeps the PE streaming one ent tile after
                    # another with the same weight chunk
                    for k in range(KCH):
                        lhsT = ansT_sb[:, k, ts(m, 128)]
                        for t in range(gsz):
                            nc.tensor.matmul(
                                pss[t][:], lhsT, ent_sb[k][:, ts(t, NTILE)],
                                start=(k == 0), stop=(k == KCH - 1))
                    for t in range(gsz):
                        copyback(out_sb[:, ts(t, NTILE)], pss[t][:])
                    # two half-width output DMAs so the drain starts as soon
                    # as the first copies land
                    h0 = (gsz // 2 + 1) * NTILE
                    nc.sync.dma_start(score[ts(m, 128), ds(col, h0)],
                                      out_sb[:, ds(0, h0)])
                    nc.sync.dma_start(score[ts(m, 128), ds(col + h0, gn - h0)],
                                      out_sb[:, ds(h0, gn - h0)])

            # fp8 DoubleRow region: K=512 as 2 matmuls of 256 (2 fp8/cell)
            col8 = gcols[-1]
            for m in range(MCH):
                out_sb = out8_pool.tile([128, NF], f16, name="out8_sb")
                last = (m == MCH - 1)
                for t in range(NF_TILES):
                    ps = psum_pool.tile([128, NTILE], f32, name="pst")
                    for j in range(2):
                        nc.tensor.matmul(
                            ps[:],
                            ans8_sb[:, ds(2 * j, 2), ts(m, 128)],
                            ent8_sb[:, ds(2 * j, 2), ds(t * NTILE, NTILE)],
                            start=(j == 0), stop=(j == 1),
                            perf_mode=DR)
                    copyback(out_sb[:, ts(t, NTILE)], ps[:])
                    if last and t % 2 == 1:
                        nc.sync.dma_start(
                            score[ts(m, 128), ds(col8 + (t - 1) * NTILE, 2 * NTILE)],
                            out_sb[:, ds((t - 1) * NTILE, 2 * NTILE)])
                if last:
                    # final tile in two halves: the kernel's tail is the last
                    # DMA's completion receipt, keep it small
                    nc.sync.dma_start(
                        score[ts(m, 128), ds(col8 + (NF_TILES - 1) * NTILE, 256)],
                        out_sb[:, ds((NF_TILES - 1) * NTILE, 256)])
                    nc.sync.dma_start(
                        score[ts(m, 128), ds(col8 + (NF_TILES - 1) * NTILE + 256, 256)],
                        out_sb[:, ds((NF_TILES - 1) * NTILE + 256, 256)])
                else:
                    h0 = 6 * NTILE
                    nc.sync.dma_start(score[ts(m, 128), ds(col8, h0)],
                                      out_sb[:, ds(0, h0)])
                    nc.sync.dma_start(score[ts(m, 128), ds(col8 + h0, NF - h0)],
                                      out_sb[:, ds(h0, NF - h0)])
            for _f in reversed(_frees):
                _f()
    nc.compile()
    return nc


def _get_nc():
    global _NC
    if _NC is None:
        _NC = _build_nc()
    return _NC


def _pmap(fn, n):
    from concurrent.futures import ThreadPoolExecutor
    with ThreadPoolExecutor(max_workers=n) as ex:
        list(ex.map(fn, range(n)))


def _to_f8_chunks(mat_t, ncols):
    """[EMB, ncols] f32 (already scaled) -> [128, KCH, ncols] e4m3 bytes."""
    q = mat_t.astype(ml_dtypes.float8_e4m3fn)
    return np.ascontiguousarray(q.reshape(KCH, 128, ncols).transpose(1, 0, 2))


def prepare_in_maps(triples, ent_emb, rel_emb):
    triples = np.asarray(triples)
    ent_emb = np.asarray(ent_emb, dtype=np.float32)
    rel_emb = np.asarray(rel_emb, dtype=np.float32)

    d = EMB // 2
    h = ent_emb[triples[:, 0].astype(np.int64)]
    r = rel_emb[triples[:, 1].astype(np.int64)]
    re_h, im_h = h[:, :d], h[:, d:]
    re_r, im_r = r[:, :d], r[:, d:]
    ans = np.empty((B, EMB), np.float32)
    ans[:, :d] = re_h * re_r - im_h * im_r
    ans[:, d:] = re_h * im_r + im_h * re_r

    ansT_bf = np.ascontiguousarray(ans.T * np.float32(OUT_SCALE)).astype(
        ml_dtypes.bfloat16)

    # fp8 scales: map absmax to ~120 (TRN e4m3 max 240), then cap the product
    # so the Cauchy-Schwarz bound on device-side scores stays inside fp16
    f8_rows = np.concatenate([
        ent_emb[c * SHARD + NB:(c + 1) * SHARD] for c in range(NCORES)])
    amax_a = float(np.abs(ans).max())
    amax_e = float(np.abs(f8_rows).max())
    s_a = 120.0 / amax_a
    s_e = 120.0 / amax_e
    cs = float(np.sqrt((ans * ans).sum(1).max()) *
               np.sqrt((f8_rows * f8_rows).sum(1).max()))
    cap = 58000.0 / cs
    if s_a * s_e > cap:
        s_a = cap / s_e
    _SCALES["fp8_inv"] = 1.0 / (s_a * s_e)

    ans8 = _to_f8_chunks(np.ascontiguousarray(ans.T) * np.float32(s_a), B)

    ent_bf = np.empty((NCORES, EMB, NB), dtype=ml_dtypes.bfloat16)
    ent8s = np.empty((NCORES, 128, KCH, NF), dtype=ml_dtypes.float8_e4m3fn)

    def _core(c):
        rows = ent_emb[c * SHARD:(c + 1) * SHARD]
        ent_bf[c] = rows[:NB].T
        blk = np.zeros((EMB, NF), np.float32)
        blk[:, :NF_REAL] = rows[NB:].T * np.float32(s_e)
        ent8s[c] = _to_f8_chunks(blk, NF)

    _pmap(_core, NCORES)
    return [{"ansT": ansT_bf, "ans8": ans8, "entT": ent_bf[c],
             "ent8": ent8s[c]} for c in range(NCORES)]


def run_raw(in_maps, trace=False):
    from concourse import bass_utils
    return bass_utils.run_bass_kernel_spmd(
        _get_nc(), in_maps, core_ids=list(range(NCORES)), trace=trace
    )


def assemble(results):
    out = np.empty((B, NUM_ENT), np.float32)
    inv16 = np.float32(1.0 / OUT_SCALE)
    inv8 = np.float32(_SCALES["fp8_inv"])

    def _one(c):
        sh = results[c]["score"]
        bf = sh[:, :NB].astype(np.float32)
        bf *= inv16
        f8 = sh[:, NB:NB + NF_REAL].astype(np.float32)
        f8 *= inv8
        out[:, c * SHARD:c * SHARD + NB] = bf
        out[:, c * SHARD + NB:(c + 1) * SHARD] = f8

    _pmap(_one, NCORES)
    return out


def kernel(triples, ent_emb, rel_emb):
    in_maps = prepare_in_maps(triples, ent_emb, rel_emb)
    res = run_raw(in_maps)
    return assemble(res.results)


# revision 12
# speedup vs baseline: 1.1488x; 1.0169x over previous
"""ComplEx scoring kernel for 8 Trainium2 NeuronCores.

Math: score[b, e] = Re(<h_b * r_b, conj(ent_e)>) with h = ent_emb[triples[:,0]],
r = rel_emb[triples[:,1]].  Writing ans_b = concat(re_h*re_r - im_h*im_r,
re_h*im_r + im_h*re_r) (shape [B, 512]), the score is exactly
score = ans @ ent_emb.T  — one [1024, 512] x [512, 200000] GEMM.

Strategy (vocab/tensor parallel on the entity axis, 25000 entities/core,
padded to 25088 = 49x512 columns):
  - host: tiny gather + complex multiply -> ans  (microseconds)
  - the GEMM is TensorE-bound (26.3 GFLOP/core vs 78.6 TF/s bf16), so the
    entity axis is split into a bf16 part (37 tiles of 512) and an fp8-e4m3
    DoubleRow part (12 tiles) that runs the PE at 2 MACs/cell/cycle.  The
    fp8 fraction (24% of entities) is sized so the fp8 quantization noise
    (3.75e-2 on those columns) keeps the global rel err ~1.8e-2 (<2e-2).
  - per core: score_bf16[1024, 18944] + score_fp8[1024, 6144], both f16,
    PE pre-warmed with dummy matmuls so the HAM clock gate opens during the
    DMA preamble instead of 3.4us into real work.
  - host: concatenate the 8 column slabs, per-region unscale, drop padding
"""

import numpy as np
import ml_dtypes

NCORES = 8
NUM_ENT = 200000
EMB = 512
B = 1024
SHARD = NUM_ENT // NCORES      # 25000 entities per core
NTILE = 512                    # matmul moving free dim == one PSUM bank
NB_TILES = 37                  # bf16 512-tiles per core
NF_TILES = 12                  # fp8 512-tiles per core
NB = NB_TILES * NTILE          # 18944 bf16 columns
NF = NF_TILES * NTILE          # 6144 fp8 columns (6056 real + 88 pad)
SHARD_PAD = NB + NF            # 25088
NF_REAL = SHARD - NB           # 6056 real entities in the fp8 region
GROUPS = [4, 3, 7, 7, 7, 7, 2] # bf16 tile groups (DMA/reuse granularity)
GN_FULL = 7 * NTILE            # 3584
KCH = EMB // 128               # 4 contraction chunks
MCH = B // 128                 # 8 batch chunks

_NC = None
_SCALES = {}

# bf16-path score values are ~1e-5 — subnormal in fp16.  Pre-scaling ans by
# 2**16 on the host puts the device-side scores in fp16's normal range; the
# host unscales.  The fp8 path has its own scales (s_a, s_e) chosen at prep
# time so quantized inputs sit in e4m3's range and scores fit fp16.
OUT_SCALE = 2.0 ** 16


def _build_nc():
    import concourse.bacc as bacc
    import concourse.bass as bass
    import concourse.tile as tile
    from concourse import mybir

    ts, ds = bass.ts, bass.ds
    bf16 = mybir.dt.bfloat16
    f16 = mybir.dt.float16
    f8 = mybir.dt.float8e4
    f32 = mybir.dt.float32
    DR = mybir.MatmulPerfMode.DoubleRow

    nc = bacc.Bacc("TRN2", target_bir_lowering=False, debug=False)
    ansT = nc.dram_tensor("ansT", [EMB, B], bf16, kind="ExternalInput")
    ans8 = nc.dram_tensor("ans8", [128, KCH, B], f8, kind="ExternalInput")
    entT = nc.dram_tensor("entT", [EMB, NB], bf16, kind="ExternalInput")
    ent8 = nc.dram_tensor("ent8", [128, KCH, NF], f8, kind="ExternalInput")
    score = nc.dram_tensor("score", [B, SHARD_PAD], f16, kind="ExternalOutput")

    with tile.TileContext(nc) as tc:
        with tc.tile_pool(name="entp", bufs=3 * KCH) as ent_pool, \
             tc.tile_pool(name="outp", bufs=4) as out_pool, \
             tc.tile_pool(name="out8p", bufs=2) as out8_pool, \
             tc.tile_pool(name="ps", bufs=8, space="PSUM") as psum_pool:

            _frees = []
            ansT_sb, _f = tc.tile([128, KCH, B], bf16, name="ansT_sb")
            _frees.append(_f)
            ans8_sb, _f = tc.tile([128, KCH, B], f8, name="ans8_sb")
            _frees.append(_f)
            ent8_sb, _f = tc.tile([128, KCH, NF], f8, name="ent8_sb")
            _frees.append(_f)
            wup, _f = tc.tile([128, 640], bf16, name="wup")
            _frees.append(_f)

            # PE pre-warm: ~4us of zero matmuls so the HAM clock gate opens
            # during the preamble/DMA wait; real matmuls then run at 2.4 GHz
            # from the first instruction.
            nc.gpsimd.memset(wup[:], 0)
            wps = psum_pool.tile([128, NTILE], f32, name="pst")
            for i in range(10):
                nc.tensor.matmul(wps[:], wup[:, ds(0, 128)],
                                 wup[:, ds(128, 512)],
                                 start=(i == 0), stop=(i == 9))

            def load_group(g, gcol):
                # one tile per k-chunk so a matmul only waits for its own DMA
                gn = GROUPS[g] * NTILE
                tiles = []
                for k in range(KCH):
                    t = ent_pool.tile([128, GN_FULL], bf16, name="ent_sb")
                    nc.sync.dma_start(t[:, ds(0, gn)],
                                      entT[ts(k, 128), ds(gcol, gn)])
                    tiles.append(t)
                return tiles

            # startup: dma_start issue costs ~650ns of SP-sequencer time
            # apiece, so use few, large DMAs.  k-slab order matches the
            # first block's k-outer consume order: the first matmul waits
            # only for ansT[k0] + the k0 ent slab (~0.8 MB).
            ent_sb0 = [ent_pool.tile([128, GN_FULL], bf16, name="ent_sb")
                       for _ in range(KCH)]
            gn0 = GROUPS[0] * NTILE
            nc.sync.dma_start(ansT_sb[:, 0], ansT[ts(0, 128), :])
            nc.sync.dma_start(ent_sb0[0][:, ds(0, gn0)],
                              entT[ts(0, 128), ds(0, gn0)])
            for k in range(1, KCH):
                nc.sync.dma_start(ansT_sb[:, k], ansT[ts(k, 128), :])
                nc.sync.dma_start(ent_sb0[k][:, ds(0, gn0)],
                                  entT[ts(k, 128), ds(0, gn0)])

            # gpsimd (Pool) cannot read PSUM on TRN2 — copyback on DVE + Act
            copy_engines = [nc.vector, nc.scalar]
            ci = 0

            def copyback(dst, ps):
                nonlocal ci
                eng = copy_engines[ci % len(copy_engines)]
                ci += 1
                if eng is nc.scalar:
                    eng.copy(dst, ps)
                else:
                    eng.tensor_copy(out=dst, in_=ps)

            ent_tiles = {0: ent_sb0}
            gcols = np.cumsum([0] + [gs * NTILE for gs in GROUPS]).tolist()
            for g, gsz in enumerate(GROUPS):
                gn = gsz * NTILE
                col = gcols[g]
                # prefetch next group's tiles ahead of this group's output
                # DMAs so the SP engine's in-order stream doesn't serialize
                if g + 1 < len(GROUPS):
                    ent_tiles[g + 1] = load_group(g + 1, gcols[g + 1])
                if g == 1:
                    # fp8 operands, needed only at the very end
                    nc.sync.dma_start(ans8_sb[:], ans8[:, :, :])
                    nc.sync.dma_start(ent8_sb[:, ds(0, 2)], ent8[:, ds(0, 2), :])
                    nc.sync.dma_start(ent8_sb[:, ds(2, 2)], ent8[:, ds(2, 2), :])
                ent_sb = ent_tiles.pop(g)

                if g == 0:
                    # warm-up: k-outer with m0+m1 interleaved (2*gsz = 8 psum
                    # banks) so each k ent slab feeds 8 matmuls (~1.8us) —
                    # faster than the ~1.5us the next slab's DMA takes, so
                    # the PE never starves while group 0 lands
                    outs = [out_pool.tile([128, GN_FULL], f16, name="out_sb")
                            for _ in range(2)]
                    pss0 = [[psum_pool.tile([128, NTILE], f32, name="pst")
                             for _ in range(gsz)] for _ in range(2)]
                    for k in range(KCH):
                        for m in range(2):
                            lhsT = ansT_sb[:, k, ts(m, 128)]
                            for t in range(gsz):
                                nc.tensor.matmul(
                                    pss0[m][t][:], lhsT,
                                    ent_sb[k][:, ts(t, NTILE)],
                                    start=(k == 0), stop=(k == KCH - 1))
                                if k == KCH - 1:
                                    copyback(outs[m][:, ts(t, NTILE)],
                                             pss0[m][t][:])
                    h0 = (gsz // 2) * NTILE
                    for m in range(2):
                        nc.sync.dma_start(score[ts(m, 128), ds(col, h0)],
                                          outs[m][:, ds(0, h0)])
                        nc.sync.dma_start(score[ts(m, 128), ds(col + h0, gn - h0)],
                                          outs[m][:, ds(h0, gn - h0)])
                    ms = range(2, MCH)
                else:
                    ms = range(MCH)

                for m in ms:
                    pss = [psum_pool.tile([128, NTILE], f32, name="pst")
                           for _ in range(gsz)]
                    out_sb = out_pool.tile([128, GN_FULL], f16, name="out_sb")
                    # k outer: keeps the PE streaming one ent tile after
                    # another with the same weight chunk
                    for k in range(KCH):
                        lhsT = ansT_sb[:, k, ts(m, 128)]
                        for t in range(gsz):
                            nc.tensor.matmul(
                                pss[t][:], lhsT, ent_sb[k][:, ts(t, NTILE)],
                                start=(k == 0), stop=(k == KCH - 1))
                    for t in range(gsz):
                        copyback(out_sb[:, ts(t, NTILE)], pss[t][:])
                    # two half-width output DMAs so the drain starts as soon
                    # as the first copies land
                    if gsz >= 4:
                        h0 = (gsz // 2 + 1) * NTILE
                        nc.sync.dma_start(score[ts(m, 128), ds(col, h0)],
                                          out_sb[:, ds(0, h0)])
                        nc.sync.dma_start(
                            score[ts(m, 128), ds(col + h0, gn - h0)],
                            out_sb[:, ds(h0, gn - h0)])
                    else:
                        nc.sync.dma_start(score[ts(m, 128), ds(col, gn)],
                                          out_sb[:, ds(0, gn)])

            # fp8 DoubleRow region: K=512 as 2 matmuls of 256 (2 fp8/cell)
            col8 = gcols[-1]
            for m in range(MCH):
                out_sb = out8_pool.tile([128, NF], f16, name="out8_sb")
                last = (m == MCH - 1)
                for t in range(NF_TILES):
                    ps = psum_pool.tile([128, NTILE], f32, name="pst")
                    for j in range(2):
                        nc.tensor.matmul(
                            ps[:],
                            ans8_sb[:, ds(2 * j, 2), ts(m, 128)],
                            ent8_sb[:, ds(2 * j, 2), ds(t * NTILE, NTILE)],
                            start=(j == 0), stop=(j == 1),
                            perf_mode=DR)
                    copyback(out_sb[:, ts(t, NTILE)], ps[:])
                    if last and t % 2 == 1:
                        nc.sync.dma_start(
                            score[ts(m, 128), ds(col8 + (t - 1) * NTILE, 2 * NTILE)],
                            out_sb[:, ds((t - 1) * NTILE, 2 * NTILE)])
                if last:
                    # final tile in two halves: the kernel's tail is the last
                    # DMA's completion receipt, keep it small
                    nc.sync.dma_start(
                        score[ts(m, 128), ds(col8 + (NF_TILES - 1) * NTILE, 256)],
                        out_sb[:, ds((NF_TILES - 1) * NTILE, 256)])
                    nc.sync.dma_start(
                        score[ts(m, 128), ds(col8 + (NF_TILES - 1) * NTILE + 256, 256)],
                        out_sb[:, ds((NF_TILES - 1) * NTILE + 256, 256)])
                else:
                    h0 = 6 * NTILE
                    nc.sync.dma_start(score[ts(m, 128), ds(col8, h0)],
                                      out_sb[:, ds(0, h0)])
                    nc.sync.dma_start(score[ts(m, 128), ds(col8 + h0, NF - h0)],
                                      out_sb[:, ds(h0, NF - h0)])
            for _f in reversed(_frees):
                _f()
    nc.compile()
    return nc


def _get_nc():
    global _NC
    if _NC is None:
        _NC = _build_nc()
    return _NC


def _pmap(fn, n):
    from concurrent.futures import ThreadPoolExecutor
    with ThreadPoolExecutor(max_workers=n) as ex:
        list(ex.map(fn, range(n)))


def _to_f8_chunks(mat_t, ncols):
    """[EMB, ncols] f32 (already scaled) -> [128, KCH, ncols] e4m3 bytes."""
    q = mat_t.astype(ml_dtypes.float8_e4m3fn)
    return np.ascontiguousarray(q.reshape(KCH, 128, ncols).transpose(1, 0, 2))


def prepare_in_maps(triples, ent_emb, rel_emb):
    triples = np.asarray(triples)
    ent_emb = np.asarray(ent_emb, dtype=np.float32)
    rel_emb = np.asarray(rel_emb, dtype=np.float32)

    d = EMB // 2
    h = ent_emb[triples[:, 0].astype(np.int64)]
    r = rel_emb[triples[:, 1].astype(np.int64)]
    re_h, im_h = h[:, :d], h[:, d:]
    re_r, im_r = r[:, :d], r[:, d:]
    ans = np.empty((B, EMB), np.float32)
    ans[:, :d] = re_h * re_r - im_h * im_r
    ans[:, d:] = re_h * im_r + im_h * re_r

    ansT_bf = np.ascontiguousarray(ans.T * np.float32(OUT_SCALE)).astype(
        ml_dtypes.bfloat16)

    # fp8 scales: map absmax to ~120 (TRN e4m3 max 240), then cap the product
    # so the Cauchy-Schwarz bound on device-side scores stays inside fp16
    f8_rows = np.concatenate([
        ent_emb[c * SHARD + NB:(c + 1) * SHARD] for c in range(NCORES)])
    amax_a = float(np.abs(ans).max())
    amax_e = float(np.abs(f8_rows).max())
    s_a = 120.0 / amax_a
    s_e = 120.0 / amax_e
    cs = float(np.sqrt((ans * ans).sum(1).max()) *
               np.sqrt((f8_rows * f8_rows).sum(1).max()))
    cap = 58000.0 / cs
    if s_a * s_e > cap:
        s_a = cap / s_e
    _SCALES["fp8_inv"] = 1.0 / (s_a * s_e)

    ans8 = _to_f8_chunks(np.ascontiguousarray(ans.T) * np.float32(s_a), B)

    ent_bf = np.empty((NCORES, EMB, NB), dtype=ml_dtypes.bfloat16)
    ent8s = np.empty((NCORES, 128, KCH, NF), dtype=ml_dtypes.float8_e4m3fn)

    def _core(c):
        rows = ent_emb[c * SHARD:(c + 1) * SHARD]
        ent_bf[c] = rows[:NB].T
        blk = np.zeros((EMB, NF), np.float32)
        blk[:, :NF_REAL] = rows[NB:].T * np.float32(s_e)
        ent8s[c] = _to_f8_chunks(blk, NF)

    _pmap(_core, NCORES)
    return [{"ansT": ansT_bf, "ans8": ans8, "entT": ent_bf[c],
             "ent8": ent8s[c]} for c in range(NCORES)]


def run_raw(in_maps, trace=False):
    from concourse import bass_utils
    return bass_utils.run_bass_kernel_spmd(
        _get_nc(), in_maps, core_ids=list(range(NCORES)), trace=trace
    )


def assemble(results):
    out = np.empty((B, NUM_ENT), np.float32)
    inv16 = np.float32(1.0 / OUT_SCALE)
    inv8 = np.float32(_SCALES["fp8_inv"])

    def _one(c):
        sh = results[c]["score"]
        bf = sh[:, :NB].astype(np.float32)
        bf *= inv16
        f8 = sh[:, NB:NB + NF_REAL].astype(np.float32)
        f8 *= inv8
        out[:, c * SHARD:c * SHARD + NB] = bf
        out[:, c * SHARD + NB:(c + 1) * SHARD] = f8

    _pmap(_one, NCORES)
    return out


def kernel(triples, ent_emb, rel_emb):
    in_maps = prepare_in_maps(triples, ent_emb, rel_emb)
    res = run_raw(in_maps)
    return assemble(res.results)
